# revision 6
# baseline (speedup 1.0000x reference)
"""Trainium2 Bass kernel for the NonLocal (non-local attention) block.

Math (per batch b, with xf = x.reshape(c, n)):
    T   = theta_w @ xf + theta_b[:, None]        # (ci, n)
    Phi = phi_w   @ xf + phi_b[:, None]          # (ci, n)
    Gt  = xf^T @ g_w^T                           # (n, ci)   (g bias folded below)
    S   = T^T @ Phi                              # (n, n)
    P   = softmax(S, axis=-1)
    Y   = Gt^T @ P^T  (normalized late by 1/rowsum(exp))      # (ci, n)
    out = W_w @ (Y + g_b 1^T) + W_b 1^T + xf
        = W_w @ Y + (x + (W_b + W_w @ g_b)[:, None])          # bias folded on host

Sharding: pure data parallel over batch; 16 batches / 8 cores = 2 per core.

Precision: projection and S matmuls use a 3-term fp16 hi/lo split
(hi*hi + hi*lo + lo*hi), which gives ~fp32-class accuracy at 3x fp16 matmul
cost. P/G/W matmuls are plain fp16 (their rounding is not amplified by the
softmax). Softmax max is exact; exp runs on the scalar engine with a fused
row-sum accumulator; normalization is applied late to the PV product via a
broadcast reciprocal row built with a tiny ones@diag matmul.
"""

import sys

if "/opt/trn_rl_repo" not in sys.path:
    sys.path.insert(0, "/opt/trn_rl_repo")

from contextlib import ExitStack

import numpy as np
import orjson

import concourse.bass as bass
import concourse.mybir as mybir
import concourse.tile as tile
from concourse.bass_utils import run_bass_kernel_spmd
from concourse.masks import make_identity

# ---------------- configuration ----------------
SPLIT = True          # 3-term fp16 split for projection + S matmuls
PT_BUFS = 2
XF_BUFS = 2
SBIG_BUFS = 2
EXP_BUFS = 2
PSUM_BUFS = 2

B, C, CI = 16, 1024, 256
HH, WW = 48, 48
NTOK = HH * WW                      # 2304
NCORES = 8
BPC = B // NCORES                   # batches per core
KO = C // 128                       # 8 c-slices
NT = NTOK // 128                    # 18 token tiles
N_CHUNKS = [(0, 512), (512, 512), (1024, 512), (1536, 512), (2048, 256)]
GROUPS = [(0, 4), (4, 4), (8, 4), (12, 4), (16, 2)]   # n_tile groups for PV

F32 = mybir.dt.float32
F16 = mybir.dt.float16

# ---------------- walrus wait-limit workaround ----------------
# This walrus build accepts only one sync-wait command per instruction
# (and none combined into an fp32/f32r Matmult's folded weight load).
# Hoist excess waits into standalone EventSemaphore instructions.
_HOIST_ALL_OPCODES = {"Matmult"}
_hoist_ctr = [0]


def _hoist_excess_waits(js):
    for f in js.get("functions", []):
        for blk in f.get("blocks", []):
            insts = blk.get("instructions", [])
            new_insts = []
            changed = False
            for i in insts:
                si = i.get("sync_info")
                waits = (si.get("on_wait") or []) if si else []
                keep = 0 if i.get("opcode") in _HOIST_ALL_OPCODES else 1
                if len(waits) > keep:
                    hoisted = waits[: len(waits) - keep]
                    kept = waits[len(waits) - keep:]
                    for w in hoisted:
                        _hoist_ctr[0] += 1
                        new_insts.append({
                            "debug": i.get("debug", 0),
                            "engine": i["engine"],
                            "ins": [],
                            "outs": [],
                            "name": f"hoistw-{_hoist_ctr[0]}",
                            "opcode": "EventSemaphore",
                            "sync_info": {"on_update": [], "on_wait": [w]},
                        })
                    si["on_wait"] = kept
                    changed = True
                new_insts.append(i)
            if changed:
                blk["instructions"] = new_insts
    return js


_orig_to_json_bytes = bass.Bass.to_json_bytes


def _patched_to_json_bytes(self):
    js = orjson.loads(_orig_to_json_bytes(self))
    _hoist_excess_waits(js)
    return orjson.dumps(js)


bass.Bass.to_json_bytes = _patched_to_json_bytes


# ---------------- kernel IR ----------------

def _emit(nc, tc, ctx, d):
    f32, f16 = F32, F16
    Ident = mybir.ActivationFunctionType.Identity
    Exp = mybir.ActivationFunctionType.Exp
    Alu = mybir.AluOpType
    AxX = mybir.AxisListType.X

    const = ctx.enter_context(tc.tile_pool(name="const", bufs=1))
    xfp = ctx.enter_context(tc.tile_pool(name="xfp", bufs=XF_BUFS))
    proj = ctx.enter_context(tc.tile_pool(name="proj", bufs=1))
    sbig = ctx.enter_context(tc.tile_pool(name="sbig", bufs=SBIG_BUFS))
    expp = ctx.enter_context(tc.tile_pool(name="expp", bufs=EXP_BUFS))
    ptp = ctx.enter_context(tc.tile_pool(name="ptp", bufs=PT_BUFS))
    rbp = ctx.enter_context(tc.tile_pool(name="rbp", bufs=1))
    stat = ctx.enter_context(tc.tile_pool(name="stat", bufs=4))
    outp = ctx.enter_context(tc.tile_pool(name="outp", bufs=3))
    psum = ctx.enter_context(tc.tile_pool(name="psum", bufs=PSUM_BUFS, space="PSUM"))
    psum1 = ctx.enter_context(tc.tile_pool(name="psum1", bufs=1, space="PSUM"))

    nsplit = 2 if SPLIT else 1
    combos = [(0, 0), (0, 1), (1, 0)] if SPLIT else [(0, 0)]

    # --- constants ---
    pw_sb = const.tile([128, KO, nsplit, 3, CI], f16, tag="pw", name="pw")
    nc.sync.dma_start(
        pw_sb[:, :, 0, :, :], d["pwh"].rearrange("(ko p) t i -> p ko t i", p=128))
    if SPLIT:
        nc.sync.dma_start(
            pw_sb[:, :, 1, :, :], d["pwl"].rearrange("(ko p) t i -> p ko t i", p=128))
    wt_sb = const.tile([128, 2, C], f16, tag="wt", name="wt")
    nc.sync.dma_start(wt_sb[:], d["wT"].rearrange("(hh p) o -> p hh o", p=128))
    tb_sb = const.tile([128, 2], f32, tag="tb", name="tb")
    nc.sync.dma_start(tb_sb[:], d["tb"].rearrange("(hh p) -> p hh", p=128))
    pb_sb = const.tile([128, 2], f32, tag="pb", name="pb")
    nc.sync.dma_start(pb_sb[:], d["pb"].rearrange("(hh p) -> p hh", p=128))

    ones_sb = const.tile([128, 128], f32, tag="ones", name="ones")
    nc.gpsimd.memset(ones_sb[:], 1.0)
    ident_sb = const.tile([128, 128], f32, tag="ident", name="ident")
    make_identity(nc, ident_sb[:])
    ident16_sb = const.tile([128, 128], f16, tag="ident16", name="ident16")
    nc.vector.tensor_copy(ident16_sb[:], ident_sb[:])

    for b in range(BPC):
        xh_b = d["xh"][b].rearrange("(ko p) n -> p ko n", p=128)
        xl_b = d["xl"][b].rearrange("(ko p) n -> p ko n", p=128) if SPLIT else None
        xr_b = d["xr"][b].rearrange("(oo p) n -> oo p n", p=128)
        out_b = d["out"][b].rearrange("(oo p) n -> oo p n", p=128)

        # persistent per-batch tiles
        th = proj.tile([128, 2, NTOK], f16, tag="th", name="th")
        phh = proj.tile([128, 2, NTOK], f16, tag="phh", name="phh")
        tl = proj.tile([128, 2, NTOK], f16, tag="tl", name="tl") if SPLIT else None
        phl = proj.tile([128, 2, NTOK], f16, tag="phl", name="phl") if SPLIT else None
        gt = proj.tile([128, NT, CI], f16, tag="gt", name="gt")
        yt = proj.tile([128, 2, NTOK], f16, tag="yt", name="yt")
        rb = rbp.tile([128, NTOK], f32, tag="rb", name="rb")

        # ---- phase A: projections ----
        for (n0, w) in N_CHUNKS:
            xt = xfp.tile([128, KO, nsplit, 512], f16, tag="xt", name="xt")
            nc.sync.dma_start(xt[:, :, 0, :w], xh_b[:, :, n0:n0 + w])
            if SPLIT:
                nc.sync.dma_start(xt[:, :, 1, :w], xl_b[:, :, n0:n0 + w])
            for pj, (dst_h, dst_l, bias_sb) in enumerate(
                    ((th, tl, tb_sb), (phh, phl, pb_sb))):
                for hh in range(2):
                    ps = psum.tile([128, 512], f32, tag="tp", name="tp")[:, :w]
                    nmm = len(combos) * KO
                    idx = 0
                    for (ws, xs) in combos:
                        for k in range(KO):
                            nc.tensor.matmul(
                                ps,
                                pw_sb[:, k, ws, pj, hh * 128:(hh + 1) * 128],
                                xt[:, k, xs, :w],
                                start=(idx == 0), stop=(idx == nmm - 1))
                            idx += 1
                    nc.scalar.activation(
                        dst_h[:, hh, n0:n0 + w], ps, Ident,
                        bias=bias_sb[:, hh:hh + 1])
                    if SPLIT:
                        nc.vector.scalar_tensor_tensor(
                            dst_l[:, hh, n0:n0 + w],
                            in0=ps,
                            scalar=bias_sb[:, hh:hh + 1],
                            in1=dst_h[:, hh, n0:n0 + w],
                            op0=Alu.add, op1=Alu.subtract)
            for mb in range(w // 128):
                psg = psum1.tile([128, CI], f32, tag="g", name="g")
                for k in range(KO):
                    nc.tensor.matmul(
                        psg,
                        xt[:, k, 0, mb * 128:(mb + 1) * 128],
                        pw_sb[:, k, 0, 2, :],
                        start=(k == 0), stop=(k == KO - 1))
                nc.vector.tensor_copy(gt[:, n0 // 128 + mb, :], psg)

        # ---- phase B: attention ----
        # Software-pipelined by one n_tile: PE transposes of tile nt are
        # emitted after the S matmuls of tile nt+1, so the PE never stalls
        # waiting for tile nt's softmax (DVE copy + max + ACT exp) chain.
        def emit_transposes(es_t, pts_t, ntl):
            for c0 in range(0, NT, 4):
                nb = min(4, NT - c0)
                ptps = psum.tile([128, 512], f16, tag="pt",
                                 name="pt")[:, :nb * 128]
                for k in range(nb):
                    nc.tensor.transpose(
                        ptps[:, k * 128:(k + 1) * 128],
                        es_t[:, (c0 + k) * 128:(c0 + k + 1) * 128],
                        ident16_sb[:])
                src = ptps.rearrange("p (b n) -> p b n", n=128)
                nc.scalar.copy(
                    pts_t[:, c0:c0 + nb, ntl * 128:(ntl + 1) * 128], src)

        def emit_pv(pts_t, t0, gn):
            gw = gn * 128
            for hh in range(2):
                psy = psum1.tile([128, 512], f32, tag="y", name="y")[:, :gw]
                for mb in range(NT):
                    nc.tensor.matmul(
                        psy,
                        gt[:, mb, hh * 128:(hh + 1) * 128],
                        pts_t[:, mb, :gw],
                        start=(mb == 0), stop=(mb == NT - 1))
                nc.vector.tensor_mul(
                    yt[:, hh, t0 * 128:t0 * 128 + gw], psy,
                    rb[:, t0 * 128:t0 * 128 + gw])

        pending = None  # (es, pts, local_idx, is_group_last, (t0, gn), pts_t)
        for (t0, gn) in GROUPS:
            pts = ptp.tile([128, NT, 512], f16, tag="pts", name="pts")
            for nt in range(t0, t0 + gn):
                ssb = sbig.tile([128, NTOK], f32, tag="ssb", name="ssb")
                for mc, (m0, mw) in enumerate(N_CHUNKS):
                    ps = psum.tile([128, 512], f32, tag="s", name="s")[:, :mw]
                    nmm = len(combos) * 2
                    idx = 0
                    for (a, bb) in combos:
                        ta = th if a == 0 else tl
                        pb_ = phh if bb == 0 else phl
                        for hh in range(2):
                            nc.tensor.matmul(
                                ps,
                                ta[:, hh, nt * 128:(nt + 1) * 128],
                                pb_[:, hh, m0:m0 + mw],
                                start=(idx == 0), stop=(idx == nmm - 1))
                            idx += 1
                    nc.vector.tensor_copy(ssb[:, m0:m0 + mw], ps)
                ngm = stat.tile([128, 1], f32, tag="ngm", name="ngm")
                nc.vector.reduce_max(ngm, ssb[:], axis=AxX, negate=True)
                es = expp.tile([128, NTOK], f16, tag="es", name="es")
                rs = stat.tile([128, 1], f32, tag="rs", name="rs")
                nc.scalar.activation(es[:], ssb[:], Exp, bias=ngm,
                                     accum_out=rs)
                rc = stat.tile([128, 1], f32, tag="rc", name="rc")
                nc.vector.reciprocal(rc, rs)
                dg = stat.tile([128, 128], f32, tag="dg", name="dg")
                nc.vector.tensor_scalar_mul(dg, ident_sb[:], rc)
                psr = psum1.tile([128, 128], f32, tag="g", name="g")
                nc.tensor.matmul(psr, ones_sb[:], dg, start=True, stop=True)
                nc.scalar.copy(rb[:, nt * 128:(nt + 1) * 128], psr)
                if pending is not None:
                    p_es, p_pts, p_ntl, p_last, p_grp = pending
                    emit_transposes(p_es, p_pts, p_ntl)
                    if p_last:
                        emit_pv(p_pts, *p_grp)
                pending = (es, pts, nt - t0, nt == t0 + gn - 1, (t0, gn))
        p_es, p_pts, p_ntl, p_last, p_grp = pending
        emit_transposes(p_es, p_pts, p_ntl)
        emit_pv(p_pts, *p_grp)

        # ---- phase C: output projection + residual ----
        for (n0, w) in N_CHUNKS:
            for oc in range(KO):
                ps = psum.tile([128, 512], f32, tag="tp", name="tp")[:, :w]
                for hh in range(2):
                    nc.tensor.matmul(
                        ps,
                        wt_sb[:, hh, oc * 128:(oc + 1) * 128],
                        yt[:, hh, n0:n0 + w],
                        start=(hh == 0), stop=(hh == 1))
                xr_t = outp.tile([128, 512], f32, tag="xr", name="xr")[:, :w]
                nc.sync.dma_start(xr_t, xr_b[oc, :, n0:n0 + w])
                ot = outp.tile([128, 512], f32, tag="ot", name="ot")[:, :w]
                nc.scalar.copy(ot, ps)
                nc.gpsimd.tensor_add(ot, ot, xr_t)
                nc.sync.dma_start(out_b[oc, :, n0:n0 + w], ot)


_nc_cache = {}


def _build():
    key = (SPLIT,)
    if key in _nc_cache:
        return _nc_cache[key]
    nc = bass.Bass(trn_type="TRN2")
    d = {}
    d["xh"] = nc.dram_tensor("xh", [BPC, C, NTOK], F16, kind="ExternalInput")
    if SPLIT:
        d["xl"] = nc.dram_tensor("xl", [BPC, C, NTOK], F16, kind="ExternalInput")
    d["xr"] = nc.dram_tensor("xr", [BPC, C, NTOK], F32, kind="ExternalInput")
    d["pwh"] = nc.dram_tensor("pwh", [C, 3, CI], F16, kind="ExternalInput")
    if SPLIT:
        d["pwl"] = nc.dram_tensor("pwl", [C, 3, CI], F16, kind="ExternalInput")
    d["wT"] = nc.dram_tensor("wT", [CI, C], F16, kind="ExternalInput")
    d["tb"] = nc.dram_tensor("tb", [CI], F32, kind="ExternalInput")
    d["pb"] = nc.dram_tensor("pb", [CI], F32, kind="ExternalInput")
    d["out"] = nc.dram_tensor("out", [BPC, C, NTOK], F32, kind="ExternalOutput")
    with ExitStack() as ctx:
        tc = ctx.enter_context(tile.TileContext(nc))
        _emit(nc, tc, ctx, d)
    _nc_cache[key] = nc
    return nc


def _prep_in_maps(x, g_w, g_b, theta_w, theta_b, phi_w, phi_b, W_w, W_b):
    x = np.asarray(x, dtype=np.float32)
    xf = x.reshape(B, C, NTOK)
    wbe = (np.asarray(W_b, np.float32)
           + np.asarray(W_w, np.float32) @ np.asarray(g_b, np.float32))
    pack = np.stack([np.asarray(theta_w, np.float32).T,
                     np.asarray(phi_w, np.float32).T,
                     np.asarray(g_w, np.float32).T], axis=1)  # (C, 3, CI)
    pwh = pack.astype(np.float16)
    pwl = (pack - pwh.astype(np.float32)).astype(np.float16)
    wT = np.asarray(W_w, np.float32).T.astype(np.float16)     # (CI, C)
    xh = xf.astype(np.float16)
    xl = (xf - xh.astype(np.float32)).astype(np.float16)
    xr = xf + wbe[None, :, None].astype(np.float32)

    in_maps = []
    for core in range(NCORES):
        sl = slice(core * BPC, (core + 1) * BPC)
        m = {
            "xh": np.ascontiguousarray(xh[sl]),
            "xr": np.ascontiguousarray(xr[sl]),
            "pwh": pwh,
            "wT": wT,
            "tb": np.asarray(theta_b, np.float32),
            "pb": np.asarray(phi_b, np.float32),
        }
        if SPLIT:
            m["xl"] = np.ascontiguousarray(xl[sl])
            m["pwl"] = pwl
        in_maps.append(m)
    return in_maps


def _run(in_maps, **kwargs):
    nc = _build()
    return run_bass_kernel_spmd(nc, in_maps, core_ids=list(range(NCORES)),
                                **kwargs)


def kernel(x, g_w, g_b, theta_w, theta_b, phi_w, phi_b, W_w, W_b):
    in_maps = _prep_in_maps(x, g_w, g_b, theta_w, theta_b, phi_w, phi_b,
                            W_w, W_b)
    res = _run(in_maps)
    outs = [r["out"].reshape(BPC, C, HH, WW) for r in res.results]
    return np.concatenate(outs, axis=0).astype(np.float32)


# revision 7
# speedup vs baseline: 1.1972x; 1.1972x over previous
"""Trainium2 Bass kernel for the NonLocal (non-local attention) block.

Math (per batch b, with xf = x.reshape(c, n)):
    T   = theta_w @ xf + theta_b[:, None]        # (ci, n)
    Phi = phi_w   @ xf + phi_b[:, None]          # (ci, n)
    Gt  = xf^T @ g_w^T                           # (n, ci)   (g bias folded below)
    S   = T^T @ Phi                              # (n, n)
    P   = softmax(S, axis=-1)
    Y   = Gt^T @ P^T  (normalized late by 1/rowsum(exp))      # (ci, n)
    out = W_w @ (Y + g_b 1^T) + W_b 1^T + xf
        = W_w @ Y + (x + (W_b + W_w @ g_b)[:, None])          # bias folded on host

Sharding: pure data parallel over batch; 16 batches / 8 cores = 2 per core.

Precision: projection and S matmuls use a 3-term fp16 hi/lo split
(hi*hi + hi*lo + lo*hi), which gives ~fp32-class accuracy at 3x fp16 matmul
cost. P/G/W matmuls are plain fp16 (their rounding is not amplified by the
softmax). Softmax max is exact; exp runs on the scalar engine with a fused
row-sum accumulator; normalization is applied late to the PV product via a
broadcast reciprocal row built with a tiny ones@diag matmul.
"""

import sys

if "/opt/trn_rl_repo" not in sys.path:
    sys.path.insert(0, "/opt/trn_rl_repo")

from contextlib import ExitStack

import numpy as np
import orjson

import concourse.bass as bass
import concourse.mybir as mybir
import concourse.tile as tile
from concourse.bass_utils import run_bass_kernel_spmd
from concourse.masks import make_identity

# ---------------- configuration ----------------
SPLIT = True          # 3-term fp16 split for projection matmuls
S_F32R = True         # store T/Phi as float32r, S matmul in f32r (1 cyc/col)
PT_BUFS = 2
XF_BUFS = 2
SBIG_BUFS = 2
EXP_BUFS = 2
PSUM_BUFS = 2

B, C, CI = 16, 1024, 256
HH, WW = 48, 48
NTOK = HH * WW                      # 2304
NCORES = 8
BPC = B // NCORES                   # batches per core
KO = C // 128                       # 8 c-slices
NT = NTOK // 128                    # 18 token tiles
N_CHUNKS = [(0, 512), (512, 512), (1024, 512), (1536, 512), (2048, 256)]
GROUPS = [(0, 4), (4, 4), (8, 4), (12, 4), (16, 2)]   # n_tile groups for PV

F32 = mybir.dt.float32
F16 = mybir.dt.float16
F32R = mybir.dt.float32r

# ---------------- walrus wait-limit workaround ----------------
# This walrus build accepts only one sync-wait command per instruction
# (and none combined into an fp32/f32r Matmult's folded weight load).
# Hoist excess waits into standalone EventSemaphore instructions.
_HOIST_ALL_OPCODES = {"Matmult"}
_hoist_ctr = [0]


def _hoist_excess_waits(js):
    for f in js.get("functions", []):
        for blk in f.get("blocks", []):
            insts = blk.get("instructions", [])
            new_insts = []
            changed = False
            for i in insts:
                si = i.get("sync_info")
                waits = (si.get("on_wait") or []) if si else []
                keep = 0 if i.get("opcode") in _HOIST_ALL_OPCODES else 1
                if len(waits) > keep:
                    hoisted = waits[: len(waits) - keep]
                    kept = waits[len(waits) - keep:]
                    for w in hoisted:
                        _hoist_ctr[0] += 1
                        new_insts.append({
                            "debug": i.get("debug", 0),
                            "engine": i["engine"],
                            "ins": [],
                            "outs": [],
                            "name": f"hoistw-{_hoist_ctr[0]}",
                            "opcode": "EventSemaphore",
                            "sync_info": {"on_update": [], "on_wait": [w]},
                        })
                    si["on_wait"] = kept
                    changed = True
                new_insts.append(i)
            if changed:
                blk["instructions"] = new_insts
    return js


_orig_to_json_bytes = bass.Bass.to_json_bytes


def _patched_to_json_bytes(self):
    js = orjson.loads(_orig_to_json_bytes(self))
    _hoist_excess_waits(js)
    return orjson.dumps(js)


bass.Bass.to_json_bytes = _patched_to_json_bytes


# ---------------- kernel IR ----------------

def _emit(nc, tc, ctx, d):
    f32, f16 = F32, F16
    Ident = mybir.ActivationFunctionType.Identity
    Exp = mybir.ActivationFunctionType.Exp
    Alu = mybir.AluOpType
    AxX = mybir.AxisListType.X

    const = ctx.enter_context(tc.tile_pool(name="const", bufs=1))
    xfp = ctx.enter_context(tc.tile_pool(name="xfp", bufs=XF_BUFS))
    proj = ctx.enter_context(tc.tile_pool(name="proj", bufs=1))
    sbig = ctx.enter_context(tc.tile_pool(name="sbig", bufs=SBIG_BUFS))
    expp = ctx.enter_context(tc.tile_pool(name="expp", bufs=EXP_BUFS))
    ptp = ctx.enter_context(tc.tile_pool(name="ptp", bufs=PT_BUFS))
    rbp = ctx.enter_context(tc.tile_pool(name="rbp", bufs=1))
    stat = ctx.enter_context(tc.tile_pool(name="stat", bufs=4))
    outp = ctx.enter_context(tc.tile_pool(name="outp", bufs=3))
    psum = ctx.enter_context(tc.tile_pool(name="psum", bufs=PSUM_BUFS, space="PSUM"))
    psum1 = ctx.enter_context(tc.tile_pool(name="psum1", bufs=1, space="PSUM"))

    nsplit = 2 if SPLIT else 1
    combos = [(0, 0), (0, 1), (1, 0)] if SPLIT else [(0, 0)]

    # --- constants ---
    pw_sb = const.tile([128, KO, nsplit, 3, CI], f16, tag="pw", name="pw")
    nc.sync.dma_start(
        pw_sb[:, :, 0, :, :], d["pwh"].rearrange("(ko p) t i -> p ko t i", p=128))
    if SPLIT:
        nc.sync.dma_start(
            pw_sb[:, :, 1, :, :], d["pwl"].rearrange("(ko p) t i -> p ko t i", p=128))
    wt_sb = const.tile([128, 2, C], f16, tag="wt", name="wt")
    nc.sync.dma_start(wt_sb[:], d["wT"].rearrange("(hh p) o -> p hh o", p=128))
    tb_sb = const.tile([128, 2], f32, tag="tb", name="tb")
    nc.sync.dma_start(tb_sb[:], d["tb"].rearrange("(hh p) -> p hh", p=128))
    pb_sb = const.tile([128, 2], f32, tag="pb", name="pb")
    nc.sync.dma_start(pb_sb[:], d["pb"].rearrange("(hh p) -> p hh", p=128))

    ones_sb = const.tile([128, 128], f32, tag="ones", name="ones")
    nc.gpsimd.memset(ones_sb[:], 1.0)
    ident_sb = const.tile([128, 128], f32, tag="ident", name="ident")
    make_identity(nc, ident_sb[:])
    ident16_sb = const.tile([128, 128], f16, tag="ident16", name="ident16")
    nc.vector.tensor_copy(ident16_sb[:], ident_sb[:])

    for b in range(BPC):
        xh_b = d["xh"][b].rearrange("(ko p) n -> p ko n", p=128)
        xl_b = d["xl"][b].rearrange("(ko p) n -> p ko n", p=128) if SPLIT else None
        xr_b = d["xr"][b].rearrange("(oo p) n -> oo p n", p=128)
        out_b = d["out"][b].rearrange("(oo p) n -> oo p n", p=128)

        # persistent per-batch tiles
        tph_dt = F32R if S_F32R else f16
        th = proj.tile([128, 2, NTOK], tph_dt, tag="th", name="th")
        phh = proj.tile([128, 2, NTOK], tph_dt, tag="phh", name="phh")
        tl = (proj.tile([128, 2, NTOK], f16, tag="tl", name="tl")
              if (SPLIT and not S_F32R) else None)
        phl = (proj.tile([128, 2, NTOK], f16, tag="phl", name="phl")
               if (SPLIT and not S_F32R) else None)
        gt = proj.tile([128, NT, CI], f16, tag="gt", name="gt")
        yt = proj.tile([128, 2, NTOK], f16, tag="yt", name="yt")
        rb = rbp.tile([128, NTOK], f32, tag="rb", name="rb")

        # ---- phase A: projections ----
        for (n0, w) in N_CHUNKS:
            xt = xfp.tile([128, KO, nsplit, 512], f16, tag="xt", name="xt")
            nc.sync.dma_start(xt[:, :, 0, :w], xh_b[:, :, n0:n0 + w])
            if SPLIT:
                nc.sync.dma_start(xt[:, :, 1, :w], xl_b[:, :, n0:n0 + w])
            for pj, (dst_h, dst_l, bias_sb) in enumerate(
                    ((th, tl, tb_sb), (phh, phl, pb_sb))):
                for hh in range(2):
                    ps = psum.tile([128, 512], f32, tag="tp", name="tp")[:, :w]
                    nmm = len(combos) * KO
                    idx = 0
                    for (ws, xs) in combos:
                        for k in range(KO):
                            nc.tensor.matmul(
                                ps,
                                pw_sb[:, k, ws, pj, hh * 128:(hh + 1) * 128],
                                xt[:, k, xs, :w],
                                start=(idx == 0), stop=(idx == nmm - 1))
                            idx += 1
                    nc.scalar.activation(
                        dst_h[:, hh, n0:n0 + w], ps, Ident,
                        bias=bias_sb[:, hh:hh + 1])
                    if SPLIT and not S_F32R:
                        nc.vector.scalar_tensor_tensor(
                            dst_l[:, hh, n0:n0 + w],
                            in0=ps,
                            scalar=bias_sb[:, hh:hh + 1],
                            in1=dst_h[:, hh, n0:n0 + w],
                            op0=Alu.add, op1=Alu.subtract)
            for mb in range(w // 128):
                psg = psum1.tile([128, CI], f32, tag="g", name="g")
                for k in range(KO):
                    nc.tensor.matmul(
                        psg,
                        xt[:, k, 0, mb * 128:(mb + 1) * 128],
                        pw_sb[:, k, 0, 2, :],
                        start=(k == 0), stop=(k == KO - 1))
                nc.vector.tensor_copy(gt[:, n0 // 128 + mb, :], psg)

        # ---- phase B: attention ----
        # Software-pipelined by one n_tile: PE transposes of tile nt are
        # emitted after the S matmuls of tile nt+1, so the PE never stalls
        # waiting for tile nt's softmax (DVE copy + max + ACT exp) chain.
        def emit_transposes(es_t, pts_t, ntl):
            for c0 in range(0, NT, 4):
                nb = min(4, NT - c0)
                ptps = psum.tile([128, 512], f16, tag="pt",
                                 name="pt")[:, :nb * 128]
                for k in range(nb):
                    nc.tensor.transpose(
                        ptps[:, k * 128:(k + 1) * 128],
                        es_t[:, (c0 + k) * 128:(c0 + k + 1) * 128],
                        ident16_sb[:])
                src = ptps.rearrange("p (b n) -> p b n", n=128)
                nc.scalar.copy(
                    pts_t[:, c0:c0 + nb, ntl * 128:(ntl + 1) * 128], src)

        def emit_pv(pts_t, t0, gn):
            gw = gn * 128
            for hh in range(2):
                psy = psum1.tile([128, 512], f32, tag="y", name="y")[:, :gw]
                for mb in range(NT):
                    nc.tensor.matmul(
                        psy,
                        gt[:, mb, hh * 128:(hh + 1) * 128],
                        pts_t[:, mb, :gw],
                        start=(mb == 0), stop=(mb == NT - 1))
                nc.vector.tensor_mul(
                    yt[:, hh, t0 * 128:t0 * 128 + gw], psy,
                    rb[:, t0 * 128:t0 * 128 + gw])

        pending = None  # (es, pts, local_idx, is_group_last, (t0, gn), pts_t)
        for (t0, gn) in GROUPS:
            pts = ptp.tile([128, NT, 512], f16, tag="pts", name="pts")
            for nt in range(t0, t0 + gn):
                ssb = sbig.tile([128, NTOK], f32, tag="ssb", name="ssb")
                s_combos = [(0, 0)] if S_F32R else combos
                for mc, (m0, mw) in enumerate(N_CHUNKS):
                    ps = psum.tile([128, 512], f32, tag="s", name="s")[:, :mw]
                    nmm = len(s_combos) * 2
                    idx = 0
                    for (a, bb) in s_combos:
                        ta = th if a == 0 else tl
                        pb_ = phh if bb == 0 else phl
                        for hh in range(2):
                            nc.tensor.matmul(
                                ps,
                                ta[:, hh, nt * 128:(nt + 1) * 128],
                                pb_[:, hh, m0:m0 + mw],
                                start=(idx == 0), stop=(idx == nmm - 1))
                            idx += 1
                    nc.vector.tensor_copy(ssb[:, m0:m0 + mw], ps)
                ngm = stat.tile([128, 1], f32, tag="ngm", name="ngm")
                nc.vector.reduce_max(ngm, ssb[:], axis=AxX, negate=True)
                es = expp.tile([128, NTOK], f16, tag="es", name="es")
                rs = stat.tile([128, 1], f32, tag="rs", name="rs")
                nc.scalar.activation(es[:], ssb[:], Exp, bias=ngm,
                                     accum_out=rs)
                rc = stat.tile([128, 1], f32, tag="rc", name="rc")
                nc.vector.reciprocal(rc, rs)
                dg = stat.tile([128, 128], f32, tag="dg", name="dg")
                nc.vector.tensor_scalar_mul(dg, ident_sb[:], rc)
                psr = psum1.tile([128, 128], f32, tag="g", name="g")
                nc.tensor.matmul(psr, ones_sb[:], dg, start=True, stop=True)
                nc.scalar.copy(rb[:, nt * 128:(nt + 1) * 128], psr)
                if pending is not None:
                    p_es, p_pts, p_ntl, p_last, p_grp = pending
                    emit_transposes(p_es, p_pts, p_ntl)
                    if p_last:
                        emit_pv(p_pts, *p_grp)
                pending = (es, pts, nt - t0, nt == t0 + gn - 1, (t0, gn))
        p_es, p_pts, p_ntl, p_last, p_grp = pending
        emit_transposes(p_es, p_pts, p_ntl)
        emit_pv(p_pts, *p_grp)

        # ---- phase C: output projection + residual ----
        for (n0, w) in N_CHUNKS:
            for oc in range(KO):
                ps = psum.tile([128, 512], f32, tag="tp", name="tp")[:, :w]
                for hh in range(2):
                    nc.tensor.matmul(
                        ps,
                        wt_sb[:, hh, oc * 128:(oc + 1) * 128],
                        yt[:, hh, n0:n0 + w],
                        start=(hh == 0), stop=(hh == 1))
                xr_t = outp.tile([128, 512], f32, tag="xr", name="xr")[:, :w]
                nc.sync.dma_start(xr_t, xr_b[oc, :, n0:n0 + w])
                ot = outp.tile([128, 512], f32, tag="ot", name="ot")[:, :w]
                nc.scalar.copy(ot, ps)
                nc.gpsimd.tensor_add(ot, ot, xr_t)
                nc.sync.dma_start(out_b[oc, :, n0:n0 + w], ot)


_nc_cache = {}


def _build():
    key = (SPLIT, S_F32R)
    if key in _nc_cache:
        return _nc_cache[key]
    nc = bass.Bass(trn_type="TRN2")
    d = {}
    d["xh"] = nc.dram_tensor("xh", [BPC, C, NTOK], F16, kind="ExternalInput")
    if SPLIT:
        d["xl"] = nc.dram_tensor("xl", [BPC, C, NTOK], F16, kind="ExternalInput")
    d["xr"] = nc.dram_tensor("xr", [BPC, C, NTOK], F32, kind="ExternalInput")
    d["pwh"] = nc.dram_tensor("pwh", [C, 3, CI], F16, kind="ExternalInput")
    if SPLIT:
        d["pwl"] = nc.dram_tensor("pwl", [C, 3, CI], F16, kind="ExternalInput")
    d["wT"] = nc.dram_tensor("wT", [CI, C], F16, kind="ExternalInput")
    d["tb"] = nc.dram_tensor("tb", [CI], F32, kind="ExternalInput")
    d["pb"] = nc.dram_tensor("pb", [CI], F32, kind="ExternalInput")
    d["out"] = nc.dram_tensor("out", [BPC, C, NTOK], F32, kind="ExternalOutput")
    with ExitStack() as ctx:
        tc = ctx.enter_context(tile.TileContext(nc))
        _emit(nc, tc, ctx, d)
    _nc_cache[key] = nc
    return nc


def _prep_in_maps(x, g_w, g_b, theta_w, theta_b, phi_w, phi_b, W_w, W_b):
    x = np.asarray(x, dtype=np.float32)
    xf = x.reshape(B, C, NTOK)
    wbe = (np.asarray(W_b, np.float32)
           + np.asarray(W_w, np.float32) @ np.asarray(g_b, np.float32))
    pack = np.stack([np.asarray(theta_w, np.float32).T,
                     np.asarray(phi_w, np.float32).T,
                     np.asarray(g_w, np.float32).T], axis=1)  # (C, 3, CI)
    pwh = pack.astype(np.float16)
    pwl = (pack - pwh.astype(np.float32)).astype(np.float16)
    wT = np.asarray(W_w, np.float32).T.astype(np.float16)     # (CI, C)
    xh = xf.astype(np.float16)
    xl = (xf - xh.astype(np.float32)).astype(np.float16)
    xr = xf + wbe[None, :, None].astype(np.float32)

    in_maps = []
    for core in range(NCORES):
        sl = slice(core * BPC, (core + 1) * BPC)
        m = {
            "xh": np.ascontiguousarray(xh[sl]),
            "xr": np.ascontiguousarray(xr[sl]),
            "pwh": pwh,
            "wT": wT,
            "tb": np.asarray(theta_b, np.float32),
            "pb": np.asarray(phi_b, np.float32),
        }
        if SPLIT:
            m["xl"] = np.ascontiguousarray(xl[sl])
            m["pwl"] = pwl
        in_maps.append(m)
    return in_maps


def _run(in_maps, **kwargs):
    nc = _build()
    return run_bass_kernel_spmd(nc, in_maps, core_ids=list(range(NCORES)),
                                **kwargs)


def kernel(x, g_w, g_b, theta_w, theta_b, phi_w, phi_b, W_w, W_b):
    in_maps = _prep_in_maps(x, g_w, g_b, theta_w, theta_b, phi_w, phi_b,
                            W_w, W_b)
    res = _run(in_maps)
    outs = [r["out"].reshape(BPC, C, HH, WW) for r in res.results]
    return np.concatenate(outs, axis=0).astype(np.float32)
